# revision 75
# baseline (speedup 1.0000x reference)
"""Trainium2 Bass kernel for DynamicPathCrossAttention.

Sharding: batch-parallel - core b computes batch element b end-to-end for its
TOP_K=2 gated paths (gating MLP evaluated on host, as it is a tiny [B,D]
computation). Weight folding on host (linear algebra around the softmax):
  G_p = Wq^T Wk_p   (logits = Q G_p S_p^T)
  H_p = Wo Wv_p     (out += w_p attn_p S_p H_p^T)

All four big [1024^3] contractions per path run as fp8e4m3 DoubleRow matmuls
(2 k-tiles per instruction, 0.5 cycles/row = 4x the fp32r rate) with 3-term
error compensation: every operand X is split host- or device-side into
X_hi = fp8(X), X_lo = fp8(X - X_hi), and the product is
  X@Y ~= X_hi@Y_hi + X_lo@Y_hi + X_hi@Y_lo     (lo@lo term dropped)
which restores ~bf16-level accuracy at 3/4 the fp32r PE time (vs 8 terms'
worth for full fp32 products). Measured end-to-end rel err ~2.6e-3.

Scale plumbing (all powers of two, host-chosen from cheap statistics):
  G' = G*sG so TMP = Q G' fits fp8 range; exp gets scale SCALE/sG on the ACT.
  A global logit shift ln_se (folded into the vb bias) guards exp overflow;
  it cancels in the softmax ratio. The gating weight ships premultiplied by
  s_a=2^9 so attention outputs land in fp8 range; H' = H^T*sH lifts H out of
  the fp8 subnormal range. The final output pass multiplies by 1/(sH*s_a)
  and adds the folded bias boe = bo + sum_p w_p Wo bv_p.

Per-path device pipeline (independent per 512-wide q-block):
  TMP[d',q]   = 3term(G', QT)        -> hi/lo split (ACT Identity + DVE sub)
  logitsT[k,q]= 3term(ST, TMP)       -> ACT Exp -> E16 (bf16) -> e_hi/e_lo
  rowsum[1,q] = ones DR matmul over e_hi+e_lo; sbc = w*s_a/rowsum broadcast
  AOS[d',q]   = 3term(SN, e)         -> *sbc (DVE) -> A16 -> a_hi/a_lo
  outT[o,q]   = 3term(H', A)         -> path0 stashes *inv+boe; path1 adds
"""

import numpy as np
import ml_dtypes

F8NP = ml_dtypes.float8_e4m3

D = 1024
P = 4
TOP_K = 2
B = 8
LQ = 1024
LK = 1024
N_CORES = 8
ND = D // 128  # 8 k-tiles of 128

_CACHE = {}


def _build_program(esc, inv):
    """esc = SCALE/sG (exp input scale); inv = 1/(sH*s_a) (output scale).
    The ones8 input arrives pre-filled with 1/s_a so the plain reciprocal
    of the rowsum directly yields sbc = s_a/rowsum."""
    import concourse.bass as bass  # noqa: F401
    import concourse.mybir as mybir
    import concourse.tile as tile
    from concourse import bacc

    f32 = mybir.dt.float32
    bf16 = mybir.dt.bfloat16
    f8 = mybir.dt.float8e4
    Exp = mybir.ActivationFunctionType.Exp
    Identity = mybir.ActivationFunctionType.Identity
    ADD = mybir.AluOpType.add
    SUB = mybir.AluOpType.subtract
    MULT = mybir.AluOpType.mult
    DR = mybir.MatmulPerfMode.DoubleRow

    SCALE = 1.0 / float(np.sqrt(D))

    nc = bacc.Bacc(
        "TRN2", target_bir_lowering=False, debug=False, enable_asserts=False
    )

    def din(name, shape, dt=f8):
        return nc.dram_tensor(name, shape, dt, kind="ExternalInput").ap()

    QT_d = [din(f"QT_{h}", [128, ND, LQ]) for h in range(2)]  # hi, lo
    G_d = [[din(f"G{j}_{h}", [128, ND, D]) for h in range(2)] for j in range(2)]
    ST_d = [[din(f"ST{j}_0", [128, ND, LK])] for j in range(2)]  # hi only
    SN_d = [[din(f"SN{j}_{h}", [128, ND, D]) for h in range(2)] for j in range(2)]
    HT_d = [[din(f"HT{j}_{h}", [128, ND, D]) for h in range(2)] for j in range(2)]
    vb_d = [din(f"vb{j}", [128, ND], f32) for j in range(2)]
    boe_d = din("boe", [128, ND], f32)
    # dual-fp8 LDWEIGHTS needs a 16B-aligned k-tile stride in the weights
    # AP, so the ones live in a [*, ND, 16] tile (8 columns used)
    ones_d = din("ones8", [128, ND, 16])
    outT = nc.dram_tensor("outT", [D, LQ], f32, kind="ExternalOutput").ap()

    with tile.TileContext(nc) as tc:
        import contextlib

        with contextlib.ExitStack() as ctx:
            const = ctx.enter_context(tc.tile_pool(name="const", bufs=1))
            qtp = ctx.enter_context(tc.tile_pool(name="qtp", bufs=1))
            wtp = ctx.enter_context(tc.tile_pool(name="wtp", bufs=1))
            tmpp = ctx.enter_context(tc.tile_pool(name="tmpp", bufs=1))
            exp_p = ctx.enter_context(tc.tile_pool(name="exp_p", bufs=1))
            aop = ctx.enter_context(tc.tile_pool(name="aop", bufs=1))
            stashp = ctx.enter_context(tc.tile_pool(name="stashp", bufs=1))
            smallp = ctx.enter_context(tc.tile_pool(name="smallp", bufs=2))
            vecp = ctx.enter_context(tc.tile_pool(name="vecp", bufs=2))
            obufp = ctx.enter_context(tc.tile_pool(name="obufp", bufs=8))
            psp = ctx.enter_context(tc.tile_pool(name="psp", bufs=8, space="PSUM"))
            dramp = ctx.enter_context(tc.tile_pool(name="dramp", bufs=2, space="DRAM"))

            # ---------------- constants ----------------
            ones8 = const.tile([128, ND, 16], f8)
            vb_t = [const.tile([128, ND], f32, name=f"vb{j}") for j in range(2)]
            boe_t = const.tile([128, ND], f32)

            def emit_const_dmas():
                nc.sync.dma_start(ones8[:], ones_d[:])
                for j in range(2):
                    nc.sync.dma_start(vb_t[j][:], vb_d[j][:])
                nc.sync.dma_start(boe_t[:], boe_d[:])

            # ---------------- resident inputs ----------------
            qt = [qtp.tile([128, ND, LQ], f8, name=f"qt{h}") for h in range(2)]

            def load_wt(j, kind, dram_pair, own_slots=False, hi_only=False):
                """Load a path stationary pair into tag-shared slots (or
                dedicated slots, avoiding the WAR wait on path0's reads)."""
                tiles = []
                for h in range(1 if hi_only else 2):
                    tag = f"{kind}{h}{j}" if own_slots else f"{kind}{h}"
                    t = wtp.tile(
                        [128, ND, D], f8, tag=tag, name=f"{kind}{h}_{j}"
                    )
                    if kind == "g" and h == 1:
                        nc.sync.dma_start(t[:, 0:6, :], dram_pair[h][:, 0:6, :])
                    else:
                        nc.sync.dma_start(t[:], dram_pair[h][:])
                    tiles.append(t)
                return tiles

            # First loads, chunked per k-pair in the exact order the first
            # (kp-major) T matmul group consumes them, so the PE starts after
            # ~1/16 of the bytes and stays fed at DMA rate.
            g = [
                wtp.tile([128, ND, D], f8, tag=f"g{h}", name=f"g{h}_0")
                for h in range(2)
            ]
            for kp in range(4):
                ks = slice(2 * kp, 2 * kp + 2)
                nc.sync.dma_start(g[0][:, ks, :], G_d[0][0][:, ks, :])
                nc.sync.dma_start(qt[0][:, ks, :], QT_d[0][:, ks, :])
                if kp < 3:  # G-lo's last k-pair is never consumed
                    nc.sync.dma_start(g[1][:, ks, :], G_d[0][1][:, ks, :])
                nc.sync.dma_start(qt[1][:, ks, :], QT_d[1][:, ks, :])
            emit_const_dmas()
            st = load_wt(0, "st", ST_d[0], hi_only=True)
            # sn0/ht0 are queued later (inside the schedule) so path-1's G
            # reaches the DMA queue before them

            # working tiles; tmp and a* double-buffered per q-block so
            # independent stages can interleave across q-blocks
            tmp = {
                qb: [
                    tmpp.tile([128, ND, 512], f8, tag=f"tmp{h}q{qb}",
                              name=f"tmp{h}q{qb}")
                    for h in range(2)
                ]
                for qb in range(2)
            }
            e16 = exp_p.tile([128, ND, 512], bf16, tag="e16", name="e16")
            eh = exp_p.tile([128, ND, 512], f8, tag="eh", name="eh")
            el = exp_p.tile([128, ND, 512], f8, tag="el", name="el")
            a16 = {
                qb: aop.tile([128, ND, 512], bf16, tag=f"a16q{qb}",
                             name=f"a16q{qb}")
                for qb in range(2)
            }
            ah = {
                qb: aop.tile([128, ND, 512], f8, tag=f"ahq{qb}", name=f"ahq{qb}")
                for qb in range(2)
            }
            al = {
                qb: aop.tile([128, ND, 512], f8, tag=f"alq{qb}", name=f"alq{qb}")
                for qb in range(2)
            }
            # path0's partial output staged in bf16 (halves SBUF; the ~0.4%
            # per-element rounding is far inside the error budget)
            stash = stashp.tile([128, ND, LQ], bf16, name="stash")

            def mm3(ps, lhs_pair, rhs_pair, lh_sl, rh_sl, skip_lhs_lo=False,
                    lhs_lo_kps=(0, 1, 2, 3)):
                """Error-compensated DR matmuls accumulating into ps, kp-major
                (defers the last-extracted k-tiles to the last instructions).
                skip_lhs_lo / lhs_lo_kps drop (some of) the lhs_lo*rhs_hi
                correction where the lhs quantization error fits the error
                budget (each full drop costs ~1.2-1.4% end-to-end vs the 2%
                gate; a half drop ~1/sqrt(2) of that).
                """
                terms = [
                    (lhs_pair[0], rhs_pair[0]),
                    (lhs_pair[1], rhs_pair[0]),
                    (lhs_pair[0], rhs_pair[1]),
                ]
                tsel = (0, 2) if skip_lhs_lo else (0, 1, 2)
                order = [
                    (t, kp)
                    for kp in range(4)
                    for t in tsel
                    if not (t == 1 and kp not in lhs_lo_kps)
                ]
                last = len(order) - 1
                for n, (t, kp) in enumerate(order):
                    ks = slice(2 * kp, 2 * kp + 2)
                    lh, rh = terms[t]
                    nc.tensor.matmul(
                        ps[:],
                        lh[:, ks, lh_sl],
                        rh[:, ks, rh_sl],
                        start=(n == 0),
                        stop=(n == last),
                        perf_mode=DR,
                    )

            # PE warmup: ramp the tensor engine to full p-state during the
            # initial DMA window with throwaway matmuls on a zeroed tile.
            warm = const.tile([128, 2, 128], f8, name="warm")
            nc.vector.memset(warm[:], 0)
            ps_w = psp.tile([128, 512], f32, tag="acc", name="ps_w")
            for _ in range(45):
                nc.tensor.matmul(
                    ps_w[:, 0:128], warm[:], warm[:],
                    start=True, stop=True, perf_mode=DR,
                )

            wts = {0: dict(g=g, st=st), 1: {}}

            def emit_T(j, qb, dts=range(8)):
                w = wts[j]["g"]
                qsl = slice(qb * 512, (qb + 1) * 512)
                for dt in dts:
                    ps = psp.tile([128, 512], f32, tag="acc", name="ps_t")
                    # G's lo-correction dropped for the last k-pair (1/4 of
                    # the contraction): ~0.7% extra error, 1 instruction
                    # fewer per group
                    mm3(ps, w, qt, slice(dt * 128, (dt + 1) * 128), qsl,
                        lhs_lo_kps=(0, 1, 2))
                    nc.scalar.activation(tmp[qb][0][:, dt, :], ps[:], Identity)
                    nc.vector.tensor_tensor(
                        tmp[qb][1][:, dt, :], ps[:], tmp[qb][0][:, dt, :], SUB
                    )

            def emit_L(j, qb):
                # ST ships hi-only: its lo-correction term is dropped (the
                # cheapest single error source, ~1.2% end-to-end vs the 2%
                # gate) - saves 1/3 of this stage's PE time and 2MB of DMA
                w = wts[j]["st"]
                for kt in range(8):
                    ps = psp.tile([128, 512], f32, tag="acc", name="ps_l")
                    mm3(ps, (w[0], None), tmp[qb],
                        slice(kt * 128, (kt + 1) * 128), slice(0, 512),
                        skip_lhs_lo=True)
                    nc.scalar.activation(
                        e16[:, kt, :],
                        ps[:],
                        Exp,
                        bias=vb_t[j][:, kt : kt + 1],
                        scale=esc,
                    )
                    nc.scalar.activation(eh[:, kt, :], e16[:, kt, :], Identity)
                    eng = nc.vector if kt % 2 == 0 else nc.gpsimd
                    eng.tensor_tensor(
                        el[:, kt, :], e16[:, kt, :], eh[:, kt, :], SUB
                    )

            def emit_rowsum(j, qb):
                """rowsum over eh+el -> sbc = s_a / rowsum on all partitions.
                Uses only PE+ACT+Pool so the DVE queue (A16 extraction) never
                waits on work queued behind it."""
                ps_s = psp.tile([8, 512], f32, tag="acc", name="ps_s")
                n = 0
                for kp in range(4):
                    ks = slice(2 * kp, 2 * kp + 2)
                    for ex in (eh, el):
                        nc.tensor.matmul(
                            ps_s[:],
                            ones8[:, ks, 0:8],
                            ex[:, ks, :],
                            start=(n == 0),
                            stop=(n == 7),
                            perf_mode=DR,
                        )
                        n += 1
                s_row = vecp.tile([1, 512], f32, tag="srow", name="s_row")
                nc.vector.reciprocal(s_row[:], ps_s[0:1, :])
                sbc = smallp.tile([128, 512], f32, tag="sbc", name="sbc")
                nc.gpsimd.partition_broadcast(sbc[:], s_row[:])
                return sbc

            def emit_A(j, qb, rs_args, split_tail=False):
                """A matmul groups with the rowsum chain emitted after the
                first two groups (their extraction waits on sbc anyway)."""
                w = wts[j]["sn"]
                sbc = None

                def extract(dt):
                    if split_tail and dt >= 6:
                        # halve the 3-hop chain latency for the last tiles
                        # (their consumer stage starts right after this one)
                        for h in range(2):
                            hs = slice(h * 256, (h + 1) * 256)
                            nc.vector.tensor_tensor(
                                a16[qb][:, dt, hs], ps_t[dt][:, hs], sbc[:, hs],
                                MULT,
                            )
                            nc.scalar.activation(
                                ah[qb][:, dt, hs], a16[qb][:, dt, hs], Identity
                            )
                            eng = nc.vector if h == 0 else nc.gpsimd
                            eng.tensor_tensor(
                                al[qb][:, dt, hs], a16[qb][:, dt, hs],
                                ah[qb][:, dt, hs], SUB,
                            )
                        return
                    nc.vector.tensor_tensor(
                        a16[qb][:, dt, :], ps_t[dt][:], sbc[:], MULT
                    )
                    nc.scalar.activation(
                        ah[qb][:, dt, :], a16[qb][:, dt, :], Identity
                    )
                    eng = nc.vector if dt % 2 == 0 else nc.gpsimd
                    eng.tensor_tensor(
                        al[qb][:, dt, :], a16[qb][:, dt, :], ah[qb][:, dt, :],
                        SUB,
                    )

                ps_t = {}
                for dt in range(8):
                    ps_t[dt] = psp.tile([128, 512], f32, tag="acc", name="ps_a")
                    mm3(ps_t[dt], w, (eh, el), slice(dt * 128, (dt + 1) * 128),
                        slice(0, 512))
                    if dt == 1:
                        sbc = emit_rowsum(*rs_args)
                        extract(0)
                    if dt >= 1:
                        extract(dt)

            def emit_O(j, qb):
                w = wts[j]["ht"]
                qsl = slice(qb * 512, (qb + 1) * 512)
                for ot in range(8):
                    ps = psp.tile([128, 512], f32, tag="acc", name="ps_o")
                    mm3(ps, w, (ah[qb], al[qb]),
                        slice(ot * 128, (ot + 1) * 128), slice(0, 512))
                    if j == 0:
                        # alternate engines so extraction keeps pace
                        if ot % 2 == 0:
                            nc.scalar.activation(
                                stash[:, ot, qsl],
                                ps[:],
                                Identity,
                                bias=boe_t[:, ot : ot + 1],
                                scale=inv,
                            )
                        else:
                            nc.vector.tensor_scalar(
                                stash[:, ot, qsl],
                                ps[:],
                                inv,
                                boe_t[:, ot : ot + 1],
                                MULT,
                                ADD,
                            )
                    else:
                        ob = obufp.tile([128, 512], f32, tag="ob", name="ob")
                        # GPSIMD cannot read PSUM; DVE owns this extraction
                        nc.vector.scalar_tensor_tensor(
                            ob[:], ps[:], inv, stash[:, ot, qsl], MULT, ADD
                        )
                        nc.sync.dma_start(
                            outT[ot * 128 : (ot + 1) * 128, qsl], ob[:]
                        )

            # Interleaved schedule: each stage boundary's extraction trail is
            # covered by an independent stage's matmuls. Ordering constraints
            # (T(qb)->L(qb)->A(qb)->O(qb) per path, buffer reuse) are enforced
            # by emission order + tile semaphores. Path-1's G gets dedicated
            # slots so its early DMA needs no WAR wait; the other path-1
            # stationaries prefetch right after their slot's last reader.
            # Both q-blocks' T groups interleaved: during the DMA-paced start
            # window each arriving chunk feeds two groups' worth of matmuls,
            # so the PE never starves while input streams in.
            for dt in range(8):
                emit_T(0, 0, dts=(dt,))
                emit_T(0, 1, dts=(dt,))
            emit_L(0, 0)
            wts[1]["g"] = load_wt(1, "g", G_d[1], own_slots=True)
            emit_T(1, 0)      # covers L(0,0) extraction for A(0,0)
            wts[0]["sn"] = load_wt(0, "sn", SN_d[0])
            emit_A(0, 0, (0, 0))
            wts[0]["ht"] = load_wt(0, "ht", HT_d[0])
            emit_L(0, 1)
            wts[1]["st"] = load_wt(1, "st", ST_d[1], hi_only=True)
            emit_O(0, 0)
            emit_A(0, 1, (0, 1))
            wts[1]["sn"] = load_wt(1, "sn", SN_d[1])
            emit_L(1, 0)      # covers path0 A(1) extraction
            emit_O(0, 1)
            wts[1]["ht"] = load_wt(1, "ht", HT_d[1])
            emit_T(1, 1)      # covers path0 O(1) extraction + out DMA
            emit_A(1, 0, (1, 0))
            emit_L(1, 1)
            emit_O(1, 0)      # covers L(1,1) extraction for A(1,1)
            emit_A(1, 1, (1, 1), split_tail=True)
            emit_O(1, 1)

    nc.compile()
    return nc


def _get_program(esc, inv):
    key = (esc, inv)
    if key not in _CACHE:
        _CACHE[key] = _build_program(esc, inv)
    return _CACHE[key]


def _host_gating(Q, Wq, bq, Wm1, bm1, Wm2, bm2):
    """Replicates the reference path-score MLP + top-k sparse weights."""
    Qm = Q.astype(np.float64).mean(axis=1)  # [B, D]
    pooled = Qm @ Wq.astype(np.float64).T + bq.astype(np.float64)
    h = np.maximum(pooled @ Wm1.astype(np.float64).T + bm1.astype(np.float64), 0.0)
    pl = h @ Wm2.astype(np.float64).T + bm2.astype(np.float64)  # [B, P]
    pl = pl - pl.max(axis=1, keepdims=True)
    e = np.exp(pl)
    scores = e / e.sum(axis=1, keepdims=True)
    idx = np.argsort(-scores, axis=1, kind="stable")[:, :TOP_K]  # [B, 2]
    w = np.take_along_axis(scores, idx, axis=1)
    wn = w / (w.sum(axis=1, keepdims=True) + 1e-8)
    return idx.astype(np.int64), wn.astype(np.float32)


def _pack(x):
    """[1024, N] contraction-major -> [128, 8, N] (partition, k-tile, free)."""
    return np.ascontiguousarray(x.reshape(ND, 128, -1).transpose(1, 0, 2))


def _split_pack(x):
    """fp8 hi/lo split then DR-pack both halves."""
    hi = x.astype(F8NP)
    lo = (x - hi.astype(np.float32)).astype(F8NP)
    return _pack(hi), _pack(lo)


def _pow2(x):
    return float(2.0 ** np.floor(np.log2(x)))


def kernel(**inputs):
    from concourse.bass_utils import run_bass_kernel_spmd

    Q = np.asarray(inputs["Q"], dtype=np.float32)
    src = np.asarray(inputs["src"], dtype=np.float32)
    Wq = np.asarray(inputs["Wq"], dtype=np.float32)
    bq = np.asarray(inputs["bq"], dtype=np.float32)
    Wk = np.asarray(inputs["Wk"], dtype=np.float32)
    bk = np.asarray(inputs["bk"], dtype=np.float32)  # noqa: F841 (cancels)
    Wv = np.asarray(inputs["Wv"], dtype=np.float32)
    bv = np.asarray(inputs["bv"], dtype=np.float32)
    Wm1 = np.asarray(inputs["Wm1"], dtype=np.float32)
    bm1 = np.asarray(inputs["bm1"], dtype=np.float32)
    Wm2 = np.asarray(inputs["Wm2"], dtype=np.float32)
    bm2 = np.asarray(inputs["bm2"], dtype=np.float32)
    Wo = np.asarray(inputs["Wo"], dtype=np.float32)
    bo = np.asarray(inputs["bo"], dtype=np.float32)

    idx, wn = _host_gating(Q, Wq, bq, Wm1, bm1, Wm2, bm2)
    SCALE = 1.0 / float(np.sqrt(D))

    sel = sorted(set(idx.flatten().tolist()))
    Gs = {p: Wq.T @ Wk[p] for p in sel}
    HTs = {p: (Wo @ Wv[p]).T for p in sel}
    g2v = {p: Wk[p].T @ bq for p in sel}
    Wobv = {p: Wo @ bv[p] for p in sel}
    vbs = {
        p: (src[p] @ g2v[p]) * SCALE if np.any(g2v[p])
        else np.zeros((B, LK), np.float32)
        for p in sel
    }

    # global power-of-two scales from cheap statistics
    sigQ = float(np.sqrt((Q**2).mean())) + 1e-30
    sigS = float(np.sqrt((src[sel] ** 2).mean())) + 1e-30
    sigT = max(
        float(np.sqrt((Gs[p] ** 2).mean() * D)) * sigQ for p in sel
    ) + 1e-30
    sG = _pow2(150.0 / (5.5 * sigT))
    sigH = max(float(np.sqrt((HTs[p] ** 2).mean())) for p in sel) + 1e-30
    sH = _pow2(2.0 / sigH)

    # exp overflow guard via a global logit shift folded into the vb bias
    # (a uniform shift of every logit cancels in the softmax ratio)
    sig_logit = sigT * sigS
    max_vb = max(float(np.abs(vbs[p]).max()) for p in sel)
    ln_se = min(0.0, float(np.log(150.0)) - (5.5 * sig_logit + max_vb))

    # attention outputs scaled by w*s_a must land in fp8 range; estimate
    # sqrt(sum attn^2) ~ e^{sig_l^2/2}/sqrt(LK) for gaussian logits
    sig_attn_out = sigS * float(np.exp(sig_logit**2 / 2)) / float(np.sqrt(LK))
    s_a = min(512.0, max(1.0, _pow2(24.0 / (5.5 * sig_attn_out))))
    inv = 1.0 / (sH * s_a)

    nc = _get_program(SCALE / sG, inv)

    Gp = {p: _split_pack(Gs[p] * sG) for p in sel}
    HTp = {p: _split_pack(HTs[p] * sH) for p in sel}
    # "ones" pre-scaled by 1/s_a (an exact power of two in fp8), so the
    # rowsum reciprocal directly yields sbc = s_a/rowsum
    ones8 = np.full((128, ND, 16), 1.0 / s_a, F8NP)

    in_maps = []
    for b in range(B):
        qh, ql = _split_pack(Q[b].T)
        m = {
            "QT_0": qh,
            "QT_1": ql,
            "ones8": ones8,
        }
        boe = bo.copy()
        for j in range(TOP_K):
            p = int(idx[b, j])
            S = src[p, b]
            sth = _pack(np.ascontiguousarray(S.T).astype(F8NP))
            # gating weight folded into the SN operand (sbc is then just
            # s_a/rowsum, a compile-time-scaled reciprocal)
            snh, snl = _split_pack(S * wn[b, j])
            m[f"G{j}_0"], m[f"G{j}_1"] = Gp[p]
            m[f"ST{j}_0"] = sth
            m[f"SN{j}_0"], m[f"SN{j}_1"] = snh, snl
            m[f"HT{j}_0"], m[f"HT{j}_1"] = HTp[p]
            vb = vbs[p][b] + ln_se
            m[f"vb{j}"] = np.ascontiguousarray(
                vb.reshape(ND, 128).T.astype(np.float32)
            )
            boe = boe + wn[b, j] * Wobv[p]
        m["boe"] = np.ascontiguousarray(boe.reshape(ND, 128).T.astype(np.float32))
        in_maps.append(m)

    res = run_bass_kernel_spmd(nc, in_maps, core_ids=list(range(N_CORES)))
    out = np.stack([res.results[b]["outT"].T for b in range(B)], axis=0)
    return np.ascontiguousarray(out).astype(np.float32)


# revision 78
# speedup vs baseline: 1.0018x; 1.0018x over previous
"""Trainium2 Bass kernel for DynamicPathCrossAttention.

Sharding: batch-parallel - core b computes batch element b end-to-end for its
TOP_K=2 gated paths (gating MLP evaluated on host, as it is a tiny [B,D]
computation). Weight folding on host (linear algebra around the softmax):
  G_p = Wq^T Wk_p   (logits = Q G_p S_p^T)
  H_p = Wo Wv_p     (out += w_p attn_p S_p H_p^T)

All four big [1024^3] contractions per path run as fp8e4m3 DoubleRow matmuls
(2 k-tiles per instruction, 0.5 cycles/row = 4x the fp32r rate) with 3-term
error compensation: every operand X is split host- or device-side into
X_hi = fp8(X), X_lo = fp8(X - X_hi), and the product is
  X@Y ~= X_hi@Y_hi + X_lo@Y_hi + X_hi@Y_lo     (lo@lo term dropped)
which restores ~bf16-level accuracy at 3/4 the fp32r PE time (vs 8 terms'
worth for full fp32 products). Measured end-to-end rel err ~2.6e-3.

Scale plumbing (all powers of two, host-chosen from cheap statistics):
  G' = G*sG so TMP = Q G' fits fp8 range; exp gets scale SCALE/sG on the ACT.
  A global logit shift ln_se (folded into the vb bias) guards exp overflow;
  it cancels in the softmax ratio. The gating weight ships premultiplied by
  s_a=2^9 so attention outputs land in fp8 range; H' = H^T*sH lifts H out of
  the fp8 subnormal range. The final output pass multiplies by 1/(sH*s_a)
  and adds the folded bias boe = bo + sum_p w_p Wo bv_p.

Per-path device pipeline (independent per 512-wide q-block):
  TMP[d',q]   = 3term(G', QT)        -> hi/lo split (ACT Identity + DVE sub)
  logitsT[k,q]= 3term(ST, TMP)       -> ACT Exp -> E16 (bf16) -> e_hi/e_lo
  rowsum[1,q] = ones DR matmul over e_hi+e_lo; sbc = w*s_a/rowsum broadcast
  AOS[d',q]   = 3term(SN, e)         -> *sbc (DVE) -> A16 -> a_hi/a_lo
  outT[o,q]   = 3term(H', A)         -> path0 stashes *inv+boe; path1 adds
"""

import numpy as np
import ml_dtypes

F8NP = ml_dtypes.float8_e4m3

D = 1024
P = 4
TOP_K = 2
B = 8
LQ = 1024
LK = 1024
N_CORES = 8
ND = D // 128  # 8 k-tiles of 128

_CACHE = {}


def _build_program(esc, inv):
    """esc = SCALE/sG (exp input scale); inv = 1/(sH*s_a) (output scale).
    The ones8 input arrives pre-filled with 1/s_a so the plain reciprocal
    of the rowsum directly yields sbc = s_a/rowsum."""
    import concourse.bass as bass  # noqa: F401
    import concourse.mybir as mybir
    import concourse.tile as tile
    from concourse import bacc

    f32 = mybir.dt.float32
    bf16 = mybir.dt.bfloat16
    f8 = mybir.dt.float8e4
    Exp = mybir.ActivationFunctionType.Exp
    Identity = mybir.ActivationFunctionType.Identity
    ADD = mybir.AluOpType.add
    SUB = mybir.AluOpType.subtract
    MULT = mybir.AluOpType.mult
    DR = mybir.MatmulPerfMode.DoubleRow

    SCALE = 1.0 / float(np.sqrt(D))

    nc = bacc.Bacc(
        "TRN2", target_bir_lowering=False, debug=False, enable_asserts=False
    )

    def din(name, shape, dt=f8):
        return nc.dram_tensor(name, shape, dt, kind="ExternalInput").ap()

    QT_d = [din(f"QT_{h}", [128, ND, LQ]) for h in range(2)]  # hi, lo
    G_d = [[din(f"G{j}_{h}", [128, ND, D]) for h in range(2)] for j in range(2)]
    ST_d = [[din(f"ST{j}_0", [128, ND, LK])] for j in range(2)]  # hi only
    SN_d = [[din(f"SN{j}_{h}", [128, ND, D]) for h in range(2)] for j in range(2)]
    HT_d = [[din(f"HT{j}_{h}", [128, ND, D]) for h in range(2)] for j in range(2)]
    vb_d = [din(f"vb{j}", [128, ND], f32) for j in range(2)]
    boe_d = din("boe", [128, ND], f32)
    # dual-fp8 LDWEIGHTS needs a 16B-aligned k-tile stride in the weights
    # AP, so the ones live in a [*, ND, 16] tile (8 columns used)
    ones_d = din("ones8", [128, ND, 16])
    outT = nc.dram_tensor("outT", [D, LQ], f32, kind="ExternalOutput").ap()

    with tile.TileContext(nc) as tc:
        import contextlib

        with contextlib.ExitStack() as ctx:
            const = ctx.enter_context(tc.tile_pool(name="const", bufs=1))
            qtp = ctx.enter_context(tc.tile_pool(name="qtp", bufs=1))
            wtp = ctx.enter_context(tc.tile_pool(name="wtp", bufs=1))
            tmpp = ctx.enter_context(tc.tile_pool(name="tmpp", bufs=1))
            exp_p = ctx.enter_context(tc.tile_pool(name="exp_p", bufs=1))
            aop = ctx.enter_context(tc.tile_pool(name="aop", bufs=1))
            stashp = ctx.enter_context(tc.tile_pool(name="stashp", bufs=1))
            smallp = ctx.enter_context(tc.tile_pool(name="smallp", bufs=2))
            vecp = ctx.enter_context(tc.tile_pool(name="vecp", bufs=2))
            obufp = ctx.enter_context(tc.tile_pool(name="obufp", bufs=8))
            psp = ctx.enter_context(tc.tile_pool(name="psp", bufs=8, space="PSUM"))
            dramp = ctx.enter_context(tc.tile_pool(name="dramp", bufs=2, space="DRAM"))

            # ---------------- constants ----------------
            ones8 = const.tile([128, ND, 16], f8)
            vb_t = [const.tile([128, ND], f32, name=f"vb{j}") for j in range(2)]
            boe_t = const.tile([128, ND], f32)

            def emit_const_dmas():
                nc.sync.dma_start(ones8[:], ones_d[:])
                for j in range(2):
                    nc.sync.dma_start(vb_t[j][:], vb_d[j][:])
                nc.sync.dma_start(boe_t[:], boe_d[:])

            # ---------------- resident inputs ----------------
            qt = [qtp.tile([128, ND, LQ], f8, name=f"qt{h}") for h in range(2)]

            def load_wt(j, kind, dram_pair, own_slots=False, hi_only=False):
                """Load a path stationary pair into tag-shared slots (or
                dedicated slots, avoiding the WAR wait on path0's reads)."""
                tiles = []
                for h in range(1 if hi_only else 2):
                    tag = f"{kind}{h}{j}" if own_slots else f"{kind}{h}"
                    t = wtp.tile(
                        [128, ND, D], f8, tag=tag, name=f"{kind}{h}_{j}"
                    )
                    if kind == "g" and h == 1:
                        nc.sync.dma_start(t[:, 0:6, :], dram_pair[h][:, 0:6, :])
                    else:
                        nc.sync.dma_start(t[:], dram_pair[h][:])
                    tiles.append(t)
                return tiles

            # First loads, chunked per k-pair in the exact order the first
            # (kp-major) T matmul group consumes them, so the PE starts after
            # ~1/16 of the bytes and stays fed at DMA rate.
            g = [
                wtp.tile([128, ND, D], f8, tag=f"g{h}", name=f"g{h}_0")
                for h in range(2)
            ]
            for kp in range(4):
                ks = slice(2 * kp, 2 * kp + 2)
                nc.sync.dma_start(g[0][:, ks, :], G_d[0][0][:, ks, :])
                nc.sync.dma_start(qt[0][:, ks, :], QT_d[0][:, ks, :])
                if kp < 3:  # G-lo's last k-pair is never consumed
                    nc.sync.dma_start(g[1][:, ks, :], G_d[0][1][:, ks, :])
                nc.sync.dma_start(qt[1][:, ks, :], QT_d[1][:, ks, :])
            emit_const_dmas()
            st = load_wt(0, "st", ST_d[0], hi_only=True)
            # sn0/ht0 are queued later (inside the schedule) so path-1's G
            # reaches the DMA queue before them

            # working tiles; tmp and a* double-buffered per q-block so
            # independent stages can interleave across q-blocks
            tmp = {
                qb: [
                    tmpp.tile([128, ND, 512], f8, tag=f"tmp{h}q{qb}",
                              name=f"tmp{h}q{qb}")
                    for h in range(2)
                ]
                for qb in range(2)
            }
            e16 = exp_p.tile([128, ND, 512], bf16, tag="e16", name="e16")
            eh = exp_p.tile([128, ND, 512], f8, tag="eh", name="eh")
            el = exp_p.tile([128, ND, 512], f8, tag="el", name="el")
            a16 = {
                qb: aop.tile([128, ND, 512], bf16, tag=f"a16q{qb}",
                             name=f"a16q{qb}")
                for qb in range(2)
            }
            ah = {
                qb: aop.tile([128, ND, 512], f8, tag=f"ahq{qb}", name=f"ahq{qb}")
                for qb in range(2)
            }
            al = {
                qb: aop.tile([128, ND, 512], f8, tag=f"alq{qb}", name=f"alq{qb}")
                for qb in range(2)
            }
            # path0's partial output staged in bf16 (halves SBUF; the ~0.4%
            # per-element rounding is far inside the error budget)
            stash = stashp.tile([128, ND, LQ], bf16, name="stash")

            def mm3(ps, lhs_pair, rhs_pair, lh_sl, rh_sl, skip_lhs_lo=False,
                    lhs_lo_kps=(0, 1, 2, 3)):
                """Error-compensated DR matmuls accumulating into ps, kp-major
                (defers the last-extracted k-tiles to the last instructions).
                skip_lhs_lo / lhs_lo_kps drop (some of) the lhs_lo*rhs_hi
                correction where the lhs quantization error fits the error
                budget (each full drop costs ~1.2-1.4% end-to-end vs the 2%
                gate; a half drop ~1/sqrt(2) of that).
                """
                terms = [
                    (lhs_pair[0], rhs_pair[0]),
                    (lhs_pair[1], rhs_pair[0]),
                    (lhs_pair[0], rhs_pair[1]),
                ]
                tsel = (0, 2) if skip_lhs_lo else (0, 1, 2)
                order = [
                    (t, kp)
                    for kp in range(4)
                    for t in tsel
                    if not (t == 1 and kp not in lhs_lo_kps)
                ]
                last = len(order) - 1
                for n, (t, kp) in enumerate(order):
                    ks = slice(2 * kp, 2 * kp + 2)
                    lh, rh = terms[t]
                    nc.tensor.matmul(
                        ps[:],
                        lh[:, ks, lh_sl],
                        rh[:, ks, rh_sl],
                        start=(n == 0),
                        stop=(n == last),
                        perf_mode=DR,
                    )

            # PE warmup: ramp the tensor engine to full p-state during the
            # initial DMA window with throwaway matmuls on a zeroed tile.
            warm = const.tile([128, 2, 128], f8, name="warm")
            nc.vector.memset(warm[:], 0)
            ps_w = psp.tile([128, 512], f32, tag="acc", name="ps_w")
            for _ in range(45):
                nc.tensor.matmul(
                    ps_w[:, 0:128], warm[:], warm[:],
                    start=True, stop=True, perf_mode=DR,
                )

            wts = {0: dict(g=g, st=st), 1: {}}

            def emit_T(j, qb, dts=range(8)):
                w = wts[j]["g"]
                qsl = slice(qb * 512, (qb + 1) * 512)
                for dt in dts:
                    ps = psp.tile([128, 512], f32, tag="acc", name="ps_t")
                    # G's lo-correction dropped for the last k-pair (1/4 of
                    # the contraction): ~0.7% extra error, 1 instruction
                    # fewer per group
                    mm3(ps, w, qt, slice(dt * 128, (dt + 1) * 128), qsl,
                        lhs_lo_kps=(0, 1, 2))
                    nc.scalar.activation(tmp[qb][0][:, dt, :], ps[:], Identity)
                    nc.vector.tensor_tensor(
                        tmp[qb][1][:, dt, :], ps[:], tmp[qb][0][:, dt, :], SUB
                    )

            def emit_L(j, qb):
                # ST ships hi-only: its lo-correction term is dropped (the
                # cheapest single error source, ~1.2% end-to-end vs the 2%
                # gate) - saves 1/3 of this stage's PE time and 2MB of DMA
                w = wts[j]["st"]
                for kt in range(8):
                    ps = psp.tile([128, 512], f32, tag="acc", name="ps_l")
                    mm3(ps, (w[0], None), tmp[qb],
                        slice(kt * 128, (kt + 1) * 128), slice(0, 512),
                        skip_lhs_lo=True)
                    nc.scalar.activation(
                        e16[:, kt, :],
                        ps[:],
                        Exp,
                        bias=vb_t[j][:, kt : kt + 1],
                        scale=esc,
                    )
                    nc.scalar.activation(eh[:, kt, :], e16[:, kt, :], Identity)
                    eng = nc.vector if kt % 2 == 0 else nc.gpsimd
                    eng.tensor_tensor(
                        el[:, kt, :], e16[:, kt, :], eh[:, kt, :], SUB
                    )

            def emit_rowsum(j, qb):
                """rowsum over eh+el -> sbc = s_a / rowsum on all partitions.
                Uses only PE+ACT+Pool so the DVE queue (A16 extraction) never
                waits on work queued behind it."""
                ps_s = psp.tile([8, 512], f32, tag="acc", name="ps_s")
                n = 0
                for kp in range(4):
                    ks = slice(2 * kp, 2 * kp + 2)
                    for ex in (eh, el):
                        nc.tensor.matmul(
                            ps_s[:],
                            ones8[:, ks, 0:8],
                            ex[:, ks, :],
                            start=(n == 0),
                            stop=(n == 7),
                            perf_mode=DR,
                        )
                        n += 1
                s_row = vecp.tile([1, 512], f32, tag="srow", name="s_row")
                nc.vector.reciprocal(s_row[:], ps_s[0:1, :])
                sbc = smallp.tile([128, 512], f32, tag="sbc", name="sbc")
                nc.gpsimd.partition_broadcast(sbc[:], s_row[:])
                return sbc

            def emit_A(j, qb, rs_args, split_tail=False):
                """A matmul groups with the rowsum chain emitted after the
                first two groups (their extraction waits on sbc anyway)."""
                w = wts[j]["sn"]
                sbc = None

                def extract(dt):
                    if split_tail and dt >= 6:
                        # halve the 3-hop chain latency for the last tiles
                        # (their consumer stage starts right after this one)
                        for h in range(2):
                            hs = slice(h * 256, (h + 1) * 256)
                            nc.vector.tensor_tensor(
                                a16[qb][:, dt, hs], ps_t[dt][:, hs], sbc[:, hs],
                                MULT,
                            )
                            nc.scalar.activation(
                                ah[qb][:, dt, hs], a16[qb][:, dt, hs], Identity
                            )
                            eng = nc.vector if h == 0 else nc.gpsimd
                            eng.tensor_tensor(
                                al[qb][:, dt, hs], a16[qb][:, dt, hs],
                                ah[qb][:, dt, hs], SUB,
                            )
                        return
                    nc.vector.tensor_tensor(
                        a16[qb][:, dt, :], ps_t[dt][:], sbc[:], MULT
                    )
                    nc.scalar.activation(
                        ah[qb][:, dt, :], a16[qb][:, dt, :], Identity
                    )
                    eng = nc.vector if dt % 2 == 0 else nc.gpsimd
                    eng.tensor_tensor(
                        al[qb][:, dt, :], a16[qb][:, dt, :], ah[qb][:, dt, :],
                        SUB,
                    )

                ps_t = {}
                for dt in range(8):
                    ps_t[dt] = psp.tile([128, 512], f32, tag="acc", name="ps_a")
                    mm3(ps_t[dt], w, (eh, el), slice(dt * 128, (dt + 1) * 128),
                        slice(0, 512))
                    if dt == 1:
                        sbc = emit_rowsum(*rs_args)
                        extract(0)
                    if dt >= 1:
                        extract(dt)

            def emit_O(j, qb):
                w = wts[j]["ht"]
                qsl = slice(qb * 512, (qb + 1) * 512)
                for ot in range(8):
                    if j == 1 and qb == 1 and ot == 7:
                        # kernel tail: run the last group as two half-width
                        # PSUM accumulations so the first half's extraction
                        # and DMA-out overlap the second half's matmuls
                        for h in range(2):
                            ps_h = psp.tile([128, 256], f32, tag="acc",
                                            name="ps_oh")
                            mm3(ps_h, w, (ah[qb], al[qb]),
                                slice(ot * 128, (ot + 1) * 128),
                                slice(h * 256, (h + 1) * 256))
                            ob = obufp.tile([128, 256], f32, tag="obh",
                                            name="obh")
                            qh = slice(qb * 512 + h * 256,
                                       qb * 512 + (h + 1) * 256)
                            nc.vector.scalar_tensor_tensor(
                                ob[:], ps_h[:], inv, stash[:, ot, qh],
                                MULT, ADD,
                            )
                            nc.sync.dma_start(
                                outT[ot * 128 : (ot + 1) * 128, qh], ob[:]
                            )
                        continue
                    ps = psp.tile([128, 512], f32, tag="acc", name="ps_o")
                    mm3(ps, w, (ah[qb], al[qb]),
                        slice(ot * 128, (ot + 1) * 128), slice(0, 512))
                    if j == 0:
                        # alternate engines so extraction keeps pace
                        if ot % 2 == 0:
                            nc.scalar.activation(
                                stash[:, ot, qsl],
                                ps[:],
                                Identity,
                                bias=boe_t[:, ot : ot + 1],
                                scale=inv,
                            )
                        else:
                            nc.vector.tensor_scalar(
                                stash[:, ot, qsl],
                                ps[:],
                                inv,
                                boe_t[:, ot : ot + 1],
                                MULT,
                                ADD,
                            )
                    else:
                        ob = obufp.tile([128, 512], f32, tag="ob", name="ob")
                        # GPSIMD cannot read PSUM; DVE owns this extraction
                        nc.vector.scalar_tensor_tensor(
                            ob[:], ps[:], inv, stash[:, ot, qsl], MULT, ADD
                        )
                        nc.sync.dma_start(
                            outT[ot * 128 : (ot + 1) * 128, qsl], ob[:]
                        )

            # Interleaved schedule: each stage boundary's extraction trail is
            # covered by an independent stage's matmuls. Ordering constraints
            # (T(qb)->L(qb)->A(qb)->O(qb) per path, buffer reuse) are enforced
            # by emission order + tile semaphores. Path-1's G gets dedicated
            # slots so its early DMA needs no WAR wait; the other path-1
            # stationaries prefetch right after their slot's last reader.
            # Both q-blocks' T groups interleaved: during the DMA-paced start
            # window each arriving chunk feeds two groups' worth of matmuls,
            # so the PE never starves while input streams in.
            for dt in range(8):
                emit_T(0, 0, dts=(dt,))
                emit_T(0, 1, dts=(dt,))
            emit_L(0, 0)
            wts[1]["g"] = load_wt(1, "g", G_d[1], own_slots=True)
            emit_T(1, 0)      # covers L(0,0) extraction for A(0,0)
            wts[0]["sn"] = load_wt(0, "sn", SN_d[0])
            emit_A(0, 0, (0, 0))
            wts[0]["ht"] = load_wt(0, "ht", HT_d[0])
            emit_L(0, 1)
            wts[1]["st"] = load_wt(1, "st", ST_d[1], hi_only=True)
            emit_O(0, 0)
            emit_A(0, 1, (0, 1))
            wts[1]["sn"] = load_wt(1, "sn", SN_d[1])
            emit_L(1, 0)      # covers path0 A(1) extraction
            emit_O(0, 1)
            wts[1]["ht"] = load_wt(1, "ht", HT_d[1])
            emit_T(1, 1)      # covers path0 O(1) extraction + out DMA
            emit_A(1, 0, (1, 0))
            emit_L(1, 1)
            emit_O(1, 0)      # covers L(1,1) extraction for A(1,1)
            emit_A(1, 1, (1, 1), split_tail=True)
            emit_O(1, 1)

    nc.compile()
    return nc


def _get_program(esc, inv):
    key = (esc, inv)
    if key not in _CACHE:
        _CACHE[key] = _build_program(esc, inv)
    return _CACHE[key]


def _host_gating(Q, Wq, bq, Wm1, bm1, Wm2, bm2):
    """Replicates the reference path-score MLP + top-k sparse weights."""
    Qm = Q.astype(np.float64).mean(axis=1)  # [B, D]
    pooled = Qm @ Wq.astype(np.float64).T + bq.astype(np.float64)
    h = np.maximum(pooled @ Wm1.astype(np.float64).T + bm1.astype(np.float64), 0.0)
    pl = h @ Wm2.astype(np.float64).T + bm2.astype(np.float64)  # [B, P]
    pl = pl - pl.max(axis=1, keepdims=True)
    e = np.exp(pl)
    scores = e / e.sum(axis=1, keepdims=True)
    idx = np.argsort(-scores, axis=1, kind="stable")[:, :TOP_K]  # [B, 2]
    w = np.take_along_axis(scores, idx, axis=1)
    wn = w / (w.sum(axis=1, keepdims=True) + 1e-8)
    return idx.astype(np.int64), wn.astype(np.float32)


def _pack(x):
    """[1024, N] contraction-major -> [128, 8, N] (partition, k-tile, free)."""
    return np.ascontiguousarray(x.reshape(ND, 128, -1).transpose(1, 0, 2))


def _split_pack(x):
    """fp8 hi/lo split then DR-pack both halves."""
    hi = x.astype(F8NP)
    lo = (x - hi.astype(np.float32)).astype(F8NP)
    return _pack(hi), _pack(lo)


def _pow2(x):
    return float(2.0 ** np.floor(np.log2(x)))


def kernel(**inputs):
    from concourse.bass_utils import run_bass_kernel_spmd

    Q = np.asarray(inputs["Q"], dtype=np.float32)
    src = np.asarray(inputs["src"], dtype=np.float32)
    Wq = np.asarray(inputs["Wq"], dtype=np.float32)
    bq = np.asarray(inputs["bq"], dtype=np.float32)
    Wk = np.asarray(inputs["Wk"], dtype=np.float32)
    bk = np.asarray(inputs["bk"], dtype=np.float32)  # noqa: F841 (cancels)
    Wv = np.asarray(inputs["Wv"], dtype=np.float32)
    bv = np.asarray(inputs["bv"], dtype=np.float32)
    Wm1 = np.asarray(inputs["Wm1"], dtype=np.float32)
    bm1 = np.asarray(inputs["bm1"], dtype=np.float32)
    Wm2 = np.asarray(inputs["Wm2"], dtype=np.float32)
    bm2 = np.asarray(inputs["bm2"], dtype=np.float32)
    Wo = np.asarray(inputs["Wo"], dtype=np.float32)
    bo = np.asarray(inputs["bo"], dtype=np.float32)

    idx, wn = _host_gating(Q, Wq, bq, Wm1, bm1, Wm2, bm2)
    SCALE = 1.0 / float(np.sqrt(D))

    sel = sorted(set(idx.flatten().tolist()))
    Gs = {p: Wq.T @ Wk[p] for p in sel}
    HTs = {p: (Wo @ Wv[p]).T for p in sel}
    g2v = {p: Wk[p].T @ bq for p in sel}
    Wobv = {p: Wo @ bv[p] for p in sel}
    vbs = {
        p: (src[p] @ g2v[p]) * SCALE if np.any(g2v[p])
        else np.zeros((B, LK), np.float32)
        for p in sel
    }

    # global power-of-two scales from cheap statistics
    sigQ = float(np.sqrt((Q**2).mean())) + 1e-30
    sigS = float(np.sqrt((src[sel] ** 2).mean())) + 1e-30
    sigT = max(
        float(np.sqrt((Gs[p] ** 2).mean() * D)) * sigQ for p in sel
    ) + 1e-30
    sG = _pow2(150.0 / (5.5 * sigT))
    sigH = max(float(np.sqrt((HTs[p] ** 2).mean())) for p in sel) + 1e-30
    sH = _pow2(2.0 / sigH)

    # exp overflow guard via a global logit shift folded into the vb bias
    # (a uniform shift of every logit cancels in the softmax ratio)
    sig_logit = sigT * sigS
    max_vb = max(float(np.abs(vbs[p]).max()) for p in sel)
    ln_se = min(0.0, float(np.log(150.0)) - (5.5 * sig_logit + max_vb))

    # attention outputs scaled by w*s_a must land in fp8 range; estimate
    # sqrt(sum attn^2) ~ e^{sig_l^2/2}/sqrt(LK) for gaussian logits
    sig_attn_out = sigS * float(np.exp(sig_logit**2 / 2)) / float(np.sqrt(LK))
    s_a = min(512.0, max(1.0, _pow2(24.0 / (5.5 * sig_attn_out))))
    inv = 1.0 / (sH * s_a)

    nc = _get_program(SCALE / sG, inv)

    Gp = {p: _split_pack(Gs[p] * sG) for p in sel}
    HTp = {p: _split_pack(HTs[p] * sH) for p in sel}
    # "ones" pre-scaled by 1/s_a (an exact power of two in fp8), so the
    # rowsum reciprocal directly yields sbc = s_a/rowsum
    ones8 = np.full((128, ND, 16), 1.0 / s_a, F8NP)

    in_maps = []
    for b in range(B):
        qh, ql = _split_pack(Q[b].T)
        m = {
            "QT_0": qh,
            "QT_1": ql,
            "ones8": ones8,
        }
        boe = bo.copy()
        for j in range(TOP_K):
            p = int(idx[b, j])
            S = src[p, b]
            sth = _pack(np.ascontiguousarray(S.T).astype(F8NP))
            # gating weight folded into the SN operand (sbc is then just
            # s_a/rowsum, a compile-time-scaled reciprocal)
            snh, snl = _split_pack(S * wn[b, j])
            m[f"G{j}_0"], m[f"G{j}_1"] = Gp[p]
            m[f"ST{j}_0"] = sth
            m[f"SN{j}_0"], m[f"SN{j}_1"] = snh, snl
            m[f"HT{j}_0"], m[f"HT{j}_1"] = HTp[p]
            vb = vbs[p][b] + ln_se
            m[f"vb{j}"] = np.ascontiguousarray(
                vb.reshape(ND, 128).T.astype(np.float32)
            )
            boe = boe + wn[b, j] * Wobv[p]
        m["boe"] = np.ascontiguousarray(boe.reshape(ND, 128).T.astype(np.float32))
        in_maps.append(m)

    res = run_bass_kernel_spmd(nc, in_maps, core_ids=list(range(N_CORES)))
    out = np.stack([res.results[b]["outT"].T for b in range(B)], axis=0)
    return np.ascontiguousarray(out).astype(np.float32)
